# revision 1
# baseline (speedup 1.0000x reference)
"""Trainium2 Bass kernel for nn_NodeAttention (gnn_message_passing).

Strategy (8 cores, data-parallel over nodes, x_1/pos_emb replicated):
  Phase 1 (per core): build a fused bf16 table T[n] = [RoPE(x_1@Wk, pos_emb[n]) | x_1@Wv]
    for ALL nodes (each core builds the full table in its own HBM).
  Phase 2 (per core, 128-node tiles of its 2500-node shard):
    - indirect-DMA gather of the 16 neighbor rows of T per node
    - q = RoPE(x_1@Wq') (Wq' pre-scaled by 1/sqrt(AFZ)), gate = sigmoid(x_1@Wg+bg)
    - scores = reduce_f(q * k_gathered) + bias2, softmax over neighbors
    - bias2 = layernorm(x_2)@Wb computed via algebraic refactor:
        LN(x)@Wb = rstd*(x@(g*Wb)) - rstd*mean*(g@Wb) + b@Wb
      with mean extracted as an extra matmul column; x_2 transposed on-chip
      via DMA-transpose (bf16) to feed the PE.
    - out = gate * sum_k(w*v); @Wback + sqrt(2)x_1 + final LN.
"""
import sys, math, os
if "/opt/trn_rl_repo" not in sys.path:
    sys.path.insert(0, "/opt/trn_rl_repo")

import numpy as np
import ml_dtypes
from contextlib import ExitStack

import concourse.bass as bass
import concourse.tile as tile
from concourse import bacc, mybir
from concourse.bass import IndirectOffsetOnAxis
from concourse.bass_utils import run_bass_kernel_spmd

P = 128
KZ, IFZ, AHZ, AFZ = 16, 256, 8, 32
HF = AHZ * AFZ  # 256
EPS = 1e-5
F32 = mybir.dt.float32
BF16 = mybir.dt.bfloat16
I32 = mybir.dt.int32
AF = mybir.ActivationFunctionType
OP = mybir.AluOpType
N_CORES = 8
N_FULL = 20000

BF = ml_dtypes.bfloat16


def build_nc(n_pad, n_shard, n_cores=N_CORES):
    """Build the SPMD Bass program. n_pad: padded full-table rows (mult of 128),
    n_shard: nodes per core (may be ragged vs 128)."""
    nt1 = n_pad // P
    nt2 = (n_shard + P - 1) // P
    n_shard_pad = nt2 * P

    nc = bacc.Bacc("TRN2", target_bir_lowering=False, debug=False,
                   num_devices=n_cores)

    # ---------------- dram I/O ----------------
    x1b = nc.dram_tensor("x1b", [n_pad, IFZ], BF16, kind="ExternalInput")
    posf = nc.dram_tensor("posf", [n_pad, 2 * AFZ], F32, kind="ExternalInput")
    x2s = nc.dram_tensor("x2s", [n_shard, KZ, IFZ], F32, kind="ExternalInput")
    eidx = nc.dram_tensor("eidx", [n_shard, KZ], I32, kind="ExternalInput")
    eidx16 = nc.dram_tensor("eidx16", [nt2 * P, P], mybir.dt.int16,
                            kind="ExternalInput")
    x1o = nc.dram_tensor("x1o", [n_shard, IFZ], F32, kind="ExternalInput")
    x1ob = nc.dram_tensor("x1ob", [n_shard_pad, IFZ], BF16, kind="ExternalInput")
    poso = nc.dram_tensor("poso", [n_shard, 2 * AFZ], F32, kind="ExternalInput")
    wq = nc.dram_tensor("wq", [IFZ, HF], F32, kind="ExternalInput")
    wk = nc.dram_tensor("wk", [IFZ, HF], F32, kind="ExternalInput")
    wv = nc.dram_tensor("wv", [IFZ, HF], F32, kind="ExternalInput")
    wg = nc.dram_tensor("wg", [IFZ, HF], F32, kind="ExternalInput")
    wb16 = nc.dram_tensor("wb16", [IFZ, 16], F32, kind="ExternalInput")
    wback = nc.dram_tensor("wback", [HF, IFZ], F32, kind="ExternalInput")
    bgv = nc.dram_tensor("bgv", [1, HF], F32, kind="ExternalInput")
    sgtb = nc.dram_tensor("sgtb", [1, 16], F32, kind="ExternalInput")
    lngb = nc.dram_tensor("lngb", [1, 2 * IFZ], F32, kind="ExternalInput")
    bbackv = nc.dram_tensor("bbackv", [1, IFZ], F32, kind="ExternalInput")
    out = nc.dram_tensor("out", [n_shard, IFZ], F32, kind="ExternalOutput")

    with tile.TileContext(nc) as tc, ExitStack() as ctx:
        const = ctx.enter_context(tc.tile_pool(name="const", bufs=1))
        dram = ctx.enter_context(tc.tile_pool(name="dram", bufs=1, space="DRAM"))
        bwork = ctx.enter_context(tc.tile_pool(name="bwork", bufs=3))
        bps = ctx.enter_context(tc.tile_pool(name="bps", bufs=2, space="PSUM"))
        work = ctx.enter_context(tc.tile_pool(name="work", bufs=2))
        psum = ctx.enter_context(tc.tile_pool(name="psum", bufs=2, space="PSUM"))

        # ---------------- constants ----------------
        wqb = const.tile([P, 2, HF], BF16)
        wkb = const.tile([P, 2, HF], BF16)
        wvb = const.tile([P, 2, HF], BF16)
        wgb = const.tile([P, 2, HF], BF16)
        wbackb = const.tile([P, 2, IFZ], BF16)
        for c in range(2):
            nc.gpsimd.dma_start(wqb[:, c, :], wq[c * P:(c + 1) * P, :])
            nc.gpsimd.dma_start(wkb[:, c, :], wk[c * P:(c + 1) * P, :])
            nc.gpsimd.dma_start(wvb[:, c, :], wv[c * P:(c + 1) * P, :])
            nc.gpsimd.dma_start(wgb[:, c, :], wg[c * P:(c + 1) * P, :])
            nc.gpsimd.dma_start(wbackb[:, c, :], wback[c * P:(c + 1) * P, :])
        wbb = const.tile([P, 2, 16], BF16)
        for c in range(2):
            nc.gpsimd.dma_start(wbb[:, c, :], wb16[c * P:(c + 1) * P, :])
        bg_r = const.tile([P, HF], F32)
        nc.sync.dma_start(bg_r[:], bgv[0:1, :].to_broadcast([P, HF]))
        sgt_r = const.tile([P, 16], F32)
        nc.sync.dma_start(sgt_r[:], sgtb[0:1, :].to_broadcast([P, 16]))
        lngb_r = const.tile([P, 2 * IFZ], F32)
        nc.sync.dma_start(lngb_r[:], lngb[0:1, :].to_broadcast([P, 2 * IFZ]))
        bback_r = const.tile([P, IFZ], F32)
        nc.sync.dma_start(bback_r[:], bbackv[0:1, :].to_broadcast([P, IFZ]))

        epsc = const.tile([P, 1], F32)
        nc.gpsimd.memset(epsc[:], EPS)

        Tt = dram.tile([n_pad, 2 * HF], BF16)

        HALF = AFZ // 2  # 16

        def rope_halves(dst_hf, src_bf16_hf, cs_t, sn_t, np_):
            """dst[(h,f)] = src*cos + rotate_half(src)*sin, all [np_, HF] bf16."""
            s_h = src_bf16_hf[:np_].rearrange("p (h f) -> p h f", h=AHZ)
            d_h = dst_hf[:np_].rearrange("p (h f) -> p h f", h=AHZ)
            t1 = bwork.tile([P, HF], BF16, tag="rope_t1")
            t1h = t1[:np_].rearrange("p (h f) -> p h f", h=AHZ)
            cs_b = cs_t[:np_, None, :].to_broadcast([np_, AHZ, AFZ])
            nc.vector.tensor_tensor(t1h, s_h, cs_b, op=OP.mult)
            t2 = bwork.tile([P, AHZ, HALF], BF16, tag="rope_t2")
            sn_lo = sn_t[:np_, None, 0:HALF].to_broadcast([np_, AHZ, HALF])
            nc.vector.tensor_tensor(t2[:np_], s_h[:, :, HALF:AFZ], sn_lo, op=OP.mult)
            nc.vector.tensor_tensor(d_h[:, :, 0:HALF], t1h[:, :, 0:HALF], t2[:np_],
                                    op=OP.subtract)
            t3 = bwork.tile([P, AHZ, HALF], BF16, tag="rope_t3")
            sn_hi = sn_t[:np_, None, HALF:AFZ].to_broadcast([np_, AHZ, HALF])
            nc.vector.tensor_tensor(t3[:np_], s_h[:, :, 0:HALF], sn_hi, op=OP.mult)
            nc.vector.tensor_tensor(d_h[:, :, HALF:AFZ], t1h[:, :, HALF:AFZ],
                                    t3[:np_], op=OP.add)

        # ---------------- phase 1: build table ----------------
        for t in range(nt1):
            x1T = bwork.tile([P, 2, P], BF16)
            nc.sync.dma_start_transpose(x1T[:], x1b[t * P:(t + 1) * P, :])
            pos_t = bwork.tile([P, 2 * AFZ], F32)
            nc.sync.dma_start(pos_t[:], posf[t * P:(t + 1) * P, :])
            kvps = bps.tile([P, 2 * HF], F32)
            kps = kvps[:, 0:HF]
            vps = kvps[:, HF:2 * HF]
            for c in range(2):
                nc.tensor.matmul(kps, x1T[:, c, :], wkb[:, c, :],
                                 start=(c == 0), stop=(c == 1))
            for c in range(2):
                nc.tensor.matmul(vps, x1T[:, c, :], wvb[:, c, :],
                                 start=(c == 0), stop=(c == 1))
            snc_t = bwork.tile([P, 2 * AFZ], BF16)
            nc.scalar.activation(snc_t[:], pos_t[:], AF.Sin)
            sn_t = snc_t[:, 0:AFZ]
            cs_t = snc_t[:, AFZ:2 * AFZ]
            kb = bwork.tile([P, HF], BF16)
            nc.scalar.copy(kb[:], kps)
            kv = bwork.tile([P, 2 * HF], BF16)
            rope_halves(kv[:, 0:HF], kb, cs_t, sn_t, P)
            nc.scalar.copy(kv[:, HF:2 * HF], vps)
            nc.sync.dma_start(Tt[t * P:(t + 1) * P, :], kv[:])

        # ---------------- phase 2: attention over own shard ----------------
        for t in range(nt2):
            np_ = min(P, n_shard - t * P)
            r0 = t * P
            full = np_ == P

            ei = work.tile([P, KZ], I32)
            nc.sync.dma_start(ei[:np_], eidx[r0:r0 + np_, :])
            q0 = (np_ // 32) * 32
            x2b = work.tile([P, KZ, IFZ], BF16, bufs=3)
            if not full:
                nc.gpsimd.memset(x2b[q0:P], 0.0)
            nc.gpsimd.dma_start(x2b[:np_], x2s[r0:r0 + np_])  # f32->bf16 cast
            x1T2 = work.tile([P, 2, P], BF16)
            nc.sync.dma_start_transpose(x1T2[:], x1ob[t * P:(t + 1) * P, :])
            pos2 = work.tile([P, 2 * AFZ], F32)
            nc.sync.dma_start(pos2[:np_], poso[r0:r0 + np_, :])
            x1r = work.tile([P, IFZ], F32)
            nc.sync.dma_start(x1r[:np_], x1o[r0:r0 + np_, :])

            kvg = work.tile([P, KZ, 2 * HF], BF16, bufs=3)
            for j in range(KZ):
                nc.gpsimd.indirect_dma_start(
                    out=kvg[:np_, j, :], out_offset=None, in_=Tt[:],
                    in_offset=IndirectOffsetOnAxis(ap=ei[:np_, j:j + 1], axis=0))

            # q and gate matmuls (share stationary x1T2 chunk)
            qgps = psum.tile([P, 2 * HF], F32)
            qps = qgps[:, 0:HF]
            gps = qgps[:, HF:2 * HF]
            for c in range(2):
                nc.tensor.matmul(qps[:np_], x1T2[:, c, :np_], wqb[:, c, :],
                                 start=(c == 0), stop=(c == 1))
            for c in range(2):
                nc.tensor.matmul(gps[:np_], x1T2[:, c, :np_], wgb[:, c, :],
                                 start=(c == 0), stop=(c == 1))

            # RoPE(q)
            snc2 = work.tile([P, 2 * AFZ], BF16)
            nc.scalar.activation(snc2[:np_], pos2[:np_], AF.Sin)
            sn2 = snc2[:, 0:AFZ]
            cs2 = snc2[:, AFZ:2 * AFZ]
            qb = work.tile([P, HF], BF16)
            nc.scalar.copy(qb[:np_], qps[:np_])
            qh = work.tile([P, HF], BF16)
            rope_halves(qh, qb, cs2, sn2, np_)

            # gate = sigmoid(gps + bg)
            gtmp = work.tile([P, HF], F32)
            nc.vector.tensor_tensor(gtmp[:np_], gps[:np_], bg_r[:np_], op=OP.add)
            gateb = work.tile([P, HF], F32)
            nc.scalar.activation(gateb[:np_], gtmp[:np_], AF.Sigmoid)

            # x2 stats: sum of squares over features (per (n,k))
            x2sq = work.tile([P, KZ, IFZ], BF16, tag="big4096")
            nc.scalar.activation(x2sq[:np_], x2b[:np_], AF.Square)
            sumsq = work.tile([P, KZ], F32)
            nc.vector.tensor_reduce(sumsq[:np_], x2sq[:np_], axis=mybir.AxisListType.X,
                                    op=OP.add)

            # x2 transpose (bf16, SBUF->SBUF DMA transpose): [f', (k,c), n]
            x2T = work.tile([P, 2 * KZ, P], BF16)
            nc.sync.dma_start_transpose(
                x2T[:], x2b[:].rearrange("p k f -> p (k f)"))
            x2Tv = x2T[:].rearrange("p (k c) n -> p c k n", c=2)

            # bias2 pre: coll[n, k, 0:8]=x2@(g*Wb), [..,8]=mean  (direct M=n matmuls)
            coll = psum.tile([P, KZ, 16], F32)
            for k in range(KZ):
                for c in range(2):
                    nc.tensor.matmul(coll[:np_, k, :], x2Tv[:, c, k, :np_],
                                     wbb[:, c, :], start=(c == 0), stop=(c == 1))

            # bias2 = rstd*(pre - mean x sg) + tb
            msq = work.tile([P, KZ], F32)
            nc.scalar.activation(msq[:np_], coll[:np_, :, 8], AF.Square)
            var = work.tile([P, KZ], F32)
            nc.vector.scalar_tensor_tensor(var[:np_], sumsq[:np_], 1.0 / IFZ,
                                           msq[:np_], op0=OP.mult, op1=OP.subtract)
            sd = work.tile([P, KZ], F32)
            nc.scalar.activation(sd[:np_], var[:np_], AF.Sqrt, bias=epsc[:np_, 0:1])
            rstd = work.tile([P, KZ], F32)
            nc.vector.reciprocal(rstd[:np_], sd[:np_])
            t1b = work.tile([P, KZ, AHZ], F32)
            nc.vector.tensor_tensor(
                t1b[:np_], coll[:np_, :, 8:9].to_broadcast([np_, KZ, AHZ]),
                sgt_r[:np_, None, 0:AHZ].to_broadcast([np_, KZ, AHZ]), op=OP.mult)
            t2b = work.tile([P, KZ, AHZ], F32)
            nc.vector.tensor_tensor(t2b[:np_], coll[:np_, :, 0:AHZ], t1b[:np_],
                                    op=OP.subtract)

            # scores = reduce_f(qh * khat) ; + bias2 terms
            prod = work.tile([P, KZ, AHZ, AFZ], BF16, tag="big4096")
            kview = kvg[:np_, :, 0:HF].rearrange("p k (h f) -> p k h f", h=AHZ)
            qbr = qh[:np_].rearrange("p (h f) -> p h f", h=AHZ)[:, None, :, :] \
                .to_broadcast([np_, KZ, AHZ, AFZ])
            nc.vector.tensor_tensor(prod[:np_], kview, qbr, op=OP.mult)
            sco = work.tile([P, KZ, AHZ], F32)
            nc.vector.tensor_reduce(sco[:np_], prod[:np_],
                                    axis=mybir.AxisListType.X, op=OP.add)
            # sco += rstd*(t2b) ... build: sco2 = sco + t2b*rstd + tb
            t3b = work.tile([P, KZ, AHZ], F32)
            nc.vector.tensor_tensor(
                t3b[:np_], t2b[:np_],
                rstd[:np_, :, None].to_broadcast([np_, KZ, AHZ]), op=OP.mult)
            nc.vector.tensor_tensor(sco[:np_], sco[:np_], t3b[:np_], op=OP.add)
            nc.vector.tensor_tensor(
                sco[:np_], sco[:np_],
                sgt_r[:np_, None, AHZ:16].to_broadcast([np_, KZ, AHZ]), op=OP.add)

            # softmax over k (unnormalized: e, rsum)
            mx = work.tile([P, AHZ], F32)
            nc.vector.tensor_reduce(mx[:np_],
                                    sco[:np_].rearrange("p k h -> p h k"),
                                    axis=mybir.AxisListType.X, op=OP.max)
            es = work.tile([P, KZ, AHZ], F32)
            nc.vector.tensor_tensor(
                es[:np_], sco[:np_],
                mx[:np_, None, :].to_broadcast([np_, KZ, AHZ]), op=OP.subtract)
            ee = work.tile([P, KZ, AHZ], BF16)
            nc.scalar.activation(ee[:np_], es[:np_], AF.Exp)
            rsum = work.tile([P, AHZ], F32)
            nc.vector.tensor_reduce(rsum[:np_],
                                    ee[:np_].rearrange("p k h -> p h k"),
                                    axis=mybir.AxisListType.X, op=OP.add)
            rinv = work.tile([P, AHZ], F32)
            nc.vector.reciprocal(rinv[:np_], rsum[:np_])

            # weighted V: wv = e*v ; tree-sum over k
            wvt = work.tile([P, KZ, AHZ, AFZ], BF16)
            vview = kvg[:np_, :, HF:2 * HF].rearrange("p k (h f) -> p k h f", h=AHZ)
            nc.vector.tensor_tensor(
                wvt[:np_], vview,
                ee[:np_, :, :, None].to_broadcast([np_, KZ, AHZ, AFZ]), op=OP.mult)
            wv8 = work.tile([P, 8, AHZ, AFZ], BF16)
            wvp = wvt[:np_].rearrange("p (k two) h f -> p k two h f", two=2)
            nc.vector.tensor_tensor(wv8[:np_], wvp[:, :, 0], wvp[:, :, 1], op=OP.add)
            wv4 = work.tile([P, 4, AHZ, AFZ], BF16)
            wvp8 = wv8[:np_].rearrange("p (k two) h f -> p k two h f", two=2)
            nc.vector.tensor_tensor(wv4[:np_], wvp8[:, :, 0], wvp8[:, :, 1], op=OP.add)
            wv2 = work.tile([P, 2, AHZ, AFZ], BF16)
            wvp4 = wv4[:np_].rearrange("p (k two) h f -> p k two h f", two=2)
            nc.vector.tensor_tensor(wv2[:np_], wvp4[:, :, 0], wvp4[:, :, 1], op=OP.add)
            att_u = work.tile([P, AHZ, AFZ], F32)
            nc.vector.tensor_tensor(att_u[:np_], wv2[:np_, 0], wv2[:np_, 1], op=OP.add)

            # att = att_u * rinv * gate  -> bf16 for back matmul
            gsc = work.tile([P, HF], F32)
            nc.vector.tensor_tensor(
                gsc[:np_].rearrange("p (h f) -> p h f", h=AHZ), gateb[:np_].rearrange("p (h f) -> p h f", h=AHZ),
                rinv[:np_, :, None].to_broadcast([np_, AHZ, AFZ]), op=OP.mult)
            att = work.tile([P, HF], BF16)
            if not full:
                nc.gpsimd.memset(att[q0:P], 0.0)
            nc.vector.tensor_tensor(att[:np_],
                                    att_u[:np_].rearrange("p h f -> p (h f)"),
                                    gsc[:np_], op=OP.mult)

            # back matmul: need attT
            attT = work.tile([P, 2, P], BF16)
            nc.sync.dma_start_transpose(attT[:], att[:])
            bps2 = psum.tile([P, IFZ], F32)
            for c in range(2):
                nc.tensor.matmul(bps2[:np_], attT[:, c, :np_], wbackb[:, c, :],
                                 start=(c == 0), stop=(c == 1))

            # residual + bback
            res = work.tile([P, IFZ], F32)
            nc.vector.scalar_tensor_tensor(res[:np_], x1r[:np_], math.sqrt(2.0),
                                           bps2[:np_], op0=OP.mult, op1=OP.add)
            nc.vector.tensor_tensor(res[:np_], res[:np_], bback_r[:np_], op=OP.add)

            # final layernorm
            smean = work.tile([P, 1], F32)
            nc.vector.tensor_reduce(smean[:np_], res[:np_],
                                    axis=mybir.AxisListType.X, op=OP.add)
            sqscr = work.tile([P, IFZ], BF16)
            sqsum = work.tile([P, 1], F32)
            nc.scalar.activation(sqscr[:np_], res[:np_], AF.Square,
                                 accum_out=sqsum[:np_])
            varf = work.tile([P, 1], F32)
            # var = sqsum/IFZ - (smean/IFZ)^2 ; compute mean first
            meanf = work.tile([P, 1], F32)
            nc.vector.tensor_scalar_mul(meanf[:np_], smean[:np_], 1.0 / IFZ)
            msqf = work.tile([P, 1], F32)
            nc.vector.tensor_tensor(msqf[:np_], meanf[:np_], meanf[:np_], op=OP.mult)
            nc.vector.scalar_tensor_tensor(varf[:np_], sqsum[:np_], 1.0 / IFZ,
                                           msqf[:np_], op0=OP.mult, op1=OP.subtract)
            sdf = work.tile([P, 1], F32)
            nc.scalar.activation(sdf[:np_], varf[:np_], AF.Sqrt, bias=epsc[:np_, 0:1])
            rstdf = work.tile([P, 1], F32)
            nc.vector.reciprocal(rstdf[:np_], sdf[:np_])
            nbias = work.tile([P, 1], F32)
            # nbias = -mean*rstd
            nc.vector.scalar_tensor_tensor(nbias[:np_], meanf[:np_], -1.0,
                                           rstdf[:np_], op0=OP.mult, op1=OP.mult)
            xn = work.tile([P, IFZ], F32)
            nc.scalar.activation(xn[:np_], res[:np_], AF.Identity,
                                 scale=rstdf[:np_], bias=nbias[:np_])
            outt = work.tile([P, IFZ], F32)
            nc.vector.tensor_tensor(outt[:np_], xn[:np_], lngb_r[:np_, 0:IFZ],
                                    op=OP.mult)
            nc.vector.tensor_tensor(outt[:np_], outt[:np_],
                                    lngb_r[:np_, IFZ:2 * IFZ], op=OP.add)
            nc.sync.dma_start(out[r0:r0 + np_, :], outt[:np_])

    nc.compile()
    return nc


_NC_CACHE = {}


def _get_nc(n_pad, n_shard, n_cores):
    key = (n_pad, n_shard, n_cores)
    if key not in _NC_CACHE:
        _NC_CACHE[key] = build_nc(n_pad, n_shard, n_cores)
    return _NC_CACHE[key]


def make_in_maps(x_1, x_2, pos_emb, edge_index, Wq, Wk, Wv, Wb, bln_g, bln_b,
                 Wg, bg, Wback, bback, ln1_g, ln1_b, n_cores=N_CORES):
    n = x_1.shape[0]
    assert n % n_cores == 0
    n_shard = n // n_cores
    nt1 = (n + P - 1) // P
    n_pad = nt1 * P
    nt2 = (n_shard + P - 1) // P
    n_shard_pad = nt2 * P

    x1b = np.zeros((n_pad, IFZ), BF)
    x1b[:n] = x_1.astype(BF)

    def red(x):
        return (x - 2 * math.pi * np.round(x / (2 * math.pi))).astype(np.float32)

    pos_sc = np.concatenate(
        [red(np.asarray(pos_emb)), red(np.asarray(pos_emb) + math.pi / 2)], axis=1)
    posf = np.zeros((n_pad, 2 * AFZ), np.float32)
    posf[:n] = pos_sc

    s = 1.0 / math.sqrt(AFZ)
    wq_s = (np.asarray(Wq) * s).astype(np.float32)
    wb16 = np.zeros((IFZ, 16), np.float32)
    wb16[:, 0:AHZ] = np.asarray(bln_g)[:, None] * np.asarray(Wb)
    wb16[:, AHZ] = 1.0 / IFZ
    sgtb = np.zeros((1, 16), np.float32)
    sgtb[0, 0:AHZ] = np.asarray(bln_g) @ np.asarray(Wb)
    sgtb[0, AHZ:2 * AHZ] = np.asarray(bln_b) @ np.asarray(Wb)
    lngb = np.concatenate([np.asarray(ln1_g), np.asarray(ln1_b)])[None, :] \
        .astype(np.float32)

    common = dict(
        x1b=x1b, posf=posf, wq=wq_s, wk=np.asarray(Wk, np.float32),
        wv=np.asarray(Wv, np.float32), wg=np.asarray(Wg, np.float32),
        wb16=wb16, wback=np.asarray(Wback, np.float32),
        bgv=np.asarray(bg, np.float32)[None, :], sgtb=sgtb, lngb=lngb,
        bbackv=np.asarray(bback, np.float32)[None, :],
    )
    in_maps = []
    for c in range(n_cores):
        lo, hi = c * n_shard, (c + 1) * n_shard
        x1ob = np.zeros((n_shard_pad, IFZ), BF)
        x1ob[:n_shard] = x_1[lo:hi].astype(BF)
        m = dict(common)
        esh = np.asarray(edge_index[lo:hi]).astype(np.int32)
        e16 = np.zeros((nt2 * P, P), np.int16)
        for t in range(nt2):
            npt = min(P, n_shard - t * P)
            if npt == P:
                flat = esh[t * P:(t + 1) * P, :].T.reshape(-1)  # e = j*128+n
                e16[t * P:t * P + 16, :] = flat.reshape(P, 16).T.astype(np.int16)
        m.update(
            x2s=np.ascontiguousarray(x_2[lo:hi], dtype=np.float32),
            eidx=esh,
            eidx16=e16,
            x1o=np.ascontiguousarray(x_1[lo:hi], dtype=np.float32),
            x1ob=x1ob,
            poso=pos_sc[lo:hi],
        )
        in_maps.append(m)
    return in_maps, n_pad, n_shard


def kernel(**inputs):
    x_1 = np.asarray(inputs["x_1"], np.float32)
    n = x_1.shape[0]
    in_maps, n_pad, n_shard = make_in_maps(**inputs)
    nc = _get_nc(n_pad, n_shard, N_CORES)
    res = run_bass_kernel_spmd(nc, in_maps, core_ids=list(range(N_CORES)),
                               trace=False)
    out = np.concatenate([res.results[c]["out"] for c in range(N_CORES)], axis=0)
    return out[:n].astype(np.float32)



# revision 12
# speedup vs baseline: 2.1639x; 2.1639x over previous
"""Trainium2 Bass kernel for nn_NodeAttention (gnn_message_passing), v2.

Strategy (8 cores, data-parallel over nodes; weights + x_1 replicated):

Phase A (per core): build bf16 table T[n] = [RoPE(x_1@Wk) | x_1@Wv] for all
  20480 (padded) nodes. Host supplies x_1 pre-transposed (feature-major) so
  the stationary loads need no on-chip transpose; 4-tile-batched DMAs; RoPE
  as 3 DVE ops using a phase-shifted sin table (one Sin activation yields
  [cos | signed-sin]); V copied out of PSUM on Act/Pool.

Phase C (per core, 20 tiles of its padded 2560-node shard): per tile
  - 16 indirect row gathers (neighbor K|V rows, 1KB each) on the SWDGE
  - q/gate matmuls (stationary = host-transposed x_1 slice)
  - bias2 = LN(x_2)@Wb via algebraic refactor:
      rstd*(x2@(g*Wb) - mean*(g@Wb)) + b@Wb
    with mean as an extra matmul column (node-stationary, using host
    feature-major x_2) and sum-of-squares via a ones-stationary matmul on
    x_2^2 whose [1,2048] transposed result is reshaped node-major by a
    small SBUF->SBUF DMA.
  - rstd via bit-trick + Newton rsqrt on DVE (no Sqrt activation table)
  - sigmoid gate via tanh (same activation table as Exp)
  - scores/softmax/weighted-V elementwise on DVE, output matmul, final LN.

Activation tables: phase A uses only Sin/Copy, phase C only
Exp/Tanh/Square/Copy/Identity -> exactly two table loads.
"""
import sys, math
if "/opt/trn_rl_repo" not in sys.path:
    sys.path.insert(0, "/opt/trn_rl_repo")

import numpy as np
import ml_dtypes
from contextlib import ExitStack

import concourse.bass as bass
import concourse.tile as tile
from concourse import bacc, mybir
from concourse.bass import IndirectOffsetOnAxis
from concourse.bass_utils import run_bass_kernel_spmd

P = 128
KZ, IFZ, AHZ, AFZ = 16, 256, 8, 32
HF = AHZ * AFZ          # 256
EPS = 1e-5
F32 = mybir.dt.float32
BF16 = mybir.dt.bfloat16
I32 = mybir.dt.int32
AF = mybir.ActivationFunctionType
OP = mybir.AluOpType
N_CORES = 8
N_FULL = 20000
NP = 20480              # padded table rows (160 tiles)
NT1 = NP // P           # 160
CH = 4                  # phase-A tiles per DMA chunk
NCH = NT1 // CH         # 40
NSH = 2560              # padded shard rows (20 tiles)
NT2 = NSH // P          # 20

BF = ml_dtypes.bfloat16
MAGIC = 0x5F3759DF


def _newton_rsqrt(nc, pool, v_ap, n_free, tag):
    """rstd = 1/sqrt(v) on DVE via bit-trick seed + 2 Newton iterations.
    v_ap: [P, n_free] f32 AP (must be a plain SBUF tile view)."""
    ti = pool.tile([P, n_free], I32, tag=f"{tag}_i")
    nc.vector.tensor_scalar(ti[:], v_ap.bitcast(I32), 1, None,
                            op0=OP.logical_shift_right)
    nc.vector.tensor_scalar(ti[:], ti[:], -1, MAGIC, op0=OP.mult, op1=OP.add)
    y = pool.tile([P, n_free], F32, tag=f"{tag}_y")
    t2 = pool.tile([P, n_free], F32, tag=f"{tag}_t")
    yf = ti[:].bitcast(F32)
    nc.vector.tensor_tensor(t2[:], yf, yf, op=OP.mult)
    nc.vector.tensor_tensor(t2[:], t2[:], v_ap, op=OP.mult)
    nc.vector.tensor_scalar(t2[:], t2[:], -0.5, 1.5, op0=OP.mult, op1=OP.add)
    nc.vector.tensor_tensor(y[:], yf, t2[:], op=OP.mult)
    nc.vector.tensor_tensor(t2[:], y[:], y[:], op=OP.mult)
    nc.vector.tensor_tensor(t2[:], t2[:], v_ap, op=OP.mult)
    nc.vector.tensor_scalar(t2[:], t2[:], -0.5, 1.5, op0=OP.mult, op1=OP.add)
    nc.vector.tensor_tensor(y[:], y[:], t2[:], op=OP.mult)
    return y


def build_nc(n_cores=N_CORES):
    nc = bacc.Bacc("TRN2", target_bir_lowering=False, debug=False,
                   num_devices=n_cores)

    # ---------------- dram I/O ----------------
    x1t = nc.dram_tensor("x1t", [2, P, NP], BF16, kind="ExternalInput")
    x1qo = nc.dram_tensor("x1qo", [P, 2, NSH], BF16, kind="ExternalInput")
    posf = nc.dram_tensor("posf", [P, NT1, 2 * AFZ], F32, kind="ExternalInput")
    poso = nc.dram_tensor("poso", [P, NT2, 2 * AFZ], F32, kind="ExternalInput")
    x2t = nc.dram_tensor("x2t", [P, 2, NT2, KZ, P], BF16, kind="ExternalInput")
    eit = nc.dram_tensor("eit", [P, NT2, KZ], I32, kind="ExternalInput")
    x1rt = nc.dram_tensor("x1rt", [P, NT2, IFZ], F32, kind="ExternalInput")
    wkv = nc.dram_tensor("wkv", [P, 2, 2 * HF], BF16, kind="ExternalInput")
    wqg = nc.dram_tensor("wqg", [P, 2, 2 * HF], BF16, kind="ExternalInput")
    wb16 = nc.dram_tensor("wb16", [P, 2, 16], BF16, kind="ExternalInput")
    wback = nc.dram_tensor("wback", [P, 2, IFZ], BF16, kind="ExternalInput")
    vecs = nc.dram_tensor("vecs", [1, 5 * IFZ + 16], F32, kind="ExternalInput")
    # vecs layout: [bg(256) | lng(256) | lnb(256) | bback(256) | x?256 unused |
    #               sg8(8) tb8(8)]
    out = nc.dram_tensor("out", [NSH, IFZ], F32, kind="ExternalOutput")

    with tile.TileContext(nc) as tc, ExitStack() as ctx:
        const = ctx.enter_context(tc.tile_pool(name="const", bufs=1))
        dram = ctx.enter_context(tc.tile_pool(name="dram", bufs=1, space="DRAM"))

        # ---------------- constants ----------------
        wkvb = const.tile([P, 2, 2 * HF], BF16)
        nc.scalar.dma_start(wkvb[:], wkv[:, :, :])
        wqgb = const.tile([P, 2, 2 * HF], BF16)
        nc.scalar.dma_start(wqgb[:], wqg[:, :, :])
        wbb = const.tile([P, 2, 16], BF16)
        nc.scalar.dma_start(wbb[:], wb16[:, :, :])
        wbackb = const.tile([P, 2, IFZ], BF16)
        nc.scalar.dma_start(wbackb[:], wback[:, :, :])
        vec_r = const.tile([P, 5 * IFZ + 16], F32)
        nc.scalar.dma_start(vec_r[:], vecs[0:1, :].to_broadcast(
            [P, 5 * IFZ + 16]))
        bg_r = vec_r[:, 0:IFZ]
        lng_r = vec_r[:, IFZ:2 * IFZ]
        lnb_r = vec_r[:, 2 * IFZ:3 * IFZ]
        bback_r = vec_r[:, 3 * IFZ:4 * IFZ]
        sg_r = vec_r[:, 5 * IFZ:5 * IFZ + 8]
        tb_r = vec_r[:, 5 * IFZ + 8:5 * IFZ + 16]
        ones1 = const.tile([P, 1], BF16)
        nc.gpsimd.memset(ones1[:], 1.0)

        Tt = dram.tile([NP, 2 * HF], BF16)

        # =============== phase A: build K|V table ===============
        with tc.tile_pool(name="apool", bufs=3) as ap, \
             tc.tile_pool(name="apsum", bufs=3, space="PSUM") as aps:
            for cc in range(NCH):
                x1c = ap.tile([P, 2, CH * P], BF16)
                nc.sync.dma_start(
                    x1c[:], x1t[:, :, cc * CH * P:(cc + 1) * CH * P]
                    .rearrange("c p n -> p c n"))
                posc = ap.tile([P, CH, 2 * AFZ], F32)
                nc.scalar.dma_start(posc[:], posf[:, cc * CH:(cc + 1) * CH, :])
                sc4 = ap.tile([P, CH, 2 * AFZ], BF16)
                nc.scalar.activation(sc4[:], posc[:], AF.Sin)
                kvo = ap.tile([P, CH, 2 * HF], BF16)
                for j in range(CH):
                    kvps = aps.tile([P, 2 * HF], F32)
                    for c in range(2):
                        nc.tensor.matmul(kvps[:], x1c[:, c, j * P:(j + 1) * P],
                                         wkvb[:, c, :], start=(c == 0),
                                         stop=(c == 1))
                    # one K|V copy to bf16 (Act); RoPE then overwrites K half
                    nc.scalar.copy(kvo[:, j, :], kvps[:])
                    cosb = sc4[:, j, None, 0:AFZ].to_broadcast([P, AHZ, AFZ])
                    ssinb = sc4[:, j, None, AFZ:2 * AFZ].to_broadcast(
                        [P, AHZ, AFZ])
                    kb = kvo[:, j, 0:HF]
                    kh = kb.rearrange("p (h f) -> p h f", h=AHZ)
                    krot = kb.rearrange("p (h two g) -> p h two g", h=AHZ,
                                        two=2)
                    ss2 = ssinb.rearrange("p h (two g) -> p h two g", two=2)
                    t1 = ap.tile([P, AHZ, AFZ], BF16, tag="t1")
                    nc.vector.tensor_tensor(t1[:], kh, cosb, op=OP.mult)
                    t23 = ap.tile([P, AHZ, 2, AFZ // 2], BF16, tag="t23")
                    nc.vector.tensor_tensor(t23[:, :, 0, :], krot[:, :, 1, :],
                                            ss2[:, :, 0, :], op=OP.mult)
                    nc.vector.tensor_tensor(t23[:, :, 1, :], krot[:, :, 0, :],
                                            ss2[:, :, 1, :], op=OP.mult)
                    nc.vector.tensor_tensor(
                        kvo[:, j, 0:HF].rearrange("p (h f) -> p h f", h=AHZ),
                        t1[:], t23[:].rearrange("p h two g -> p h (two g)"),
                        op=OP.add)
                nc.sync.dma_start(
                    Tt[cc * CH * P:(cc + 1) * CH * P, :]
                    .rearrange("(j p) f -> p j f", p=P), kvo[:])

            # own-shard sincos (still Sin table)
            sc_own = const.tile([P, NT2, 2 * AFZ], BF16)
            for q in range(NT2 // CH):
                po = ap.tile([P, CH, 2 * AFZ], F32, tag="po")
                nc.scalar.dma_start(po[:], poso[:, q * CH:(q + 1) * CH, :])
                nc.scalar.activation(sc_own[:, q * CH:(q + 1) * CH, :], po[:],
                                     AF.Sin)

        # =============== phase C: attention over own shard ===============
        with tc.tile_pool(name="cpool", bufs=2) as cp, \
             tc.tile_pool(name="cgath", bufs=2) as cg, \
             tc.tile_pool(name="cpsum", bufs=2, space="PSUM") as cps, \
             tc.tile_pool(name="cpsum1", bufs=1, space="PSUM") as cps1:
            eis = const.tile([P, NT2, KZ], I32)
            nc.sync.dma_start(eis[:], eit[:, :, :])

            for t in range(NT2):
                t4 = t % CH
                if t4 == 0:
                    x1q = cp.tile([P, 2, CH * P], BF16, tag="x1q")
                    nc.sync.dma_start(x1q[:],
                                      x1qo[:, :, t * P:(t + CH) * P])
                    x1rc = cp.tile([P, CH, IFZ], F32, tag="x1rc")
                    nc.scalar.dma_start(x1rc[:],
                                        x1rt[:, t:t + CH, :])
                    outw = cp.tile([P, CH, IFZ], F32, tag="outw")

                # ---- DMAs for this tile
                x2tt = cp.tile([P, 2, KZ, P], BF16, tag="x2tt")
                nc.scalar.dma_start(x2tt[:], x2t[:, :, t, :, :])
                kvg = cg.tile([P, KZ, 2 * HF], BF16, tag="kvg")
                for j in range(KZ):
                    nc.gpsimd.indirect_dma_start(
                        out=kvg[:, j, :], out_offset=None, in_=Tt[:],
                        in_offset=IndirectOffsetOnAxis(
                            ap=eis[:, t, j:j + 1], axis=0))

                # ---- q/gate matmuls
                qg = cps.tile([P, 2 * HF], F32, tag="qg")
                for c in range(2):
                    nc.tensor.matmul(qg[:], x1q[:, c, t4 * P:(t4 + 1) * P],
                                     wqgb[:, c, :], start=(c == 0),
                                     stop=(c == 1))

                # ---- bias2 pre: coll[n, k, 0:8]=x2@(g*Wb), [n,k,8]=mean
                coll = cps.tile([P, KZ, 16], F32, tag="coll")
                for k in range(KZ):
                    for c in range(2):
                        nc.tensor.matmul(coll[:, k, :], x2tt[:, c, k, :],
                                         wbb[:, c, :], start=(c == 0),
                                         stop=(c == 1))

                # ---- sum of squares via ones-stationary matmul
                x2sq = cp.tile([P, 2, KZ, P], BF16, tag="x2sq")
                nc.scalar.activation(x2sq[:], x2tt[:], AF.Square)
                # chunk q (n-group) -> psum row {0,32}[q%2], bank half q//2
                ssT = cps1.tile([33, 2 * 2 * HF], F32, tag="ssT")
                x2v = x2sq[:].rearrange("p c k n -> p c n k")
                for q in range(4):
                    r, b = 32 * (q % 2), 2 * HF * (q // 2)
                    for c in range(2):
                        nc.tensor.matmul(
                            ssT[r:r + 1, b:b + 2 * HF],
                            ones1[:], x2v[:, c, 32 * q:32 * (q + 1), :],
                            start=(c == 0), stop=(c == 1))
                sst_sb = cp.tile([33, 2 * 2 * HF], F32, tag="sst_sb")
                nc.scalar.copy(sst_sb[0:1, :], ssT[0:1, :])
                nc.scalar.copy(sst_sb[32:33, :], ssT[32:33, :])
                ssq = cp.tile([P, KZ], F32, tag="ssq")
                for q in range(4):
                    r, b = 32 * (q % 2), 2 * HF * (q // 2)
                    nc.scalar.dma_start(
                        ssq[32 * q:32 * (q + 1), :]
                        .rearrange("p (o k) -> p o k", o=1),
                        sst_sb[r:r + 1, b:b + 2 * HF]
                        .rearrange("o (n k) -> o n k", n=32))

                # ---- RoPE(q) (reads qg PSUM f32)
                qh = cp.tile([P, AHZ, AFZ], BF16, tag="qh")
                cosb = sc_own[:, t, None, 0:AFZ].to_broadcast([P, AHZ, AFZ])
                ssinb = sc_own[:, t, None, AFZ:2 * AFZ].to_broadcast(
                    [P, AHZ, AFZ])
                qv = qg[:, 0:HF].rearrange("p (h f) -> p h f", h=AHZ)
                qrot = qg[:, 0:HF].rearrange("p (h two g) -> p h two g",
                                             h=AHZ, two=2)
                tq1 = cp.tile([P, AHZ, AFZ], BF16, tag="tq1")
                nc.vector.tensor_tensor(tq1[:], qv, cosb, op=OP.mult)
                tq2 = cp.tile([P, AHZ, 2, AFZ // 2], BF16, tag="tq2")
                ss2 = ssinb.rearrange("p h (two g) -> p h two g", two=2)
                nc.vector.tensor_tensor(tq2[:, :, 0, :], qrot[:, :, 1, :],
                                        ss2[:, :, 0, :], op=OP.mult)
                nc.vector.tensor_tensor(tq2[:, :, 1, :], qrot[:, :, 0, :],
                                        ss2[:, :, 1, :], op=OP.mult)
                nc.vector.tensor_tensor(
                    qh[:], tq1[:], tq2[:].rearrange("p h two g -> p h (two g)"),
                    op=OP.add)

                # ---- gate = sigmoid(x) = 0.5*tanh(0.5x)+0.5
                xg = cp.tile([P, HF], F32, tag="xg")
                nc.vector.tensor_tensor(xg[:], qg[:, HF:2 * HF], bg_r,
                                        op=OP.add)
                th = cp.tile([P, HF], BF16, tag="th")
                nc.scalar.activation(th[:], xg[:], AF.Tanh, scale=0.5)
                gate = cp.tile([P, HF], BF16, tag="gate")
                nc.vector.tensor_scalar(gate[:], th[:], 0.5, 0.5, op0=OP.mult,
                                        op1=OP.add)

                # ---- scores
                kview = kvg[:, :, 0:HF].rearrange("p k (h f) -> p k h f",
                                                  h=AHZ)
                qb = qh[:, None, :, :].to_broadcast([P, KZ, AHZ, AFZ])
                prod = cp.tile([P, KZ, AHZ, AFZ], BF16, tag="prod")
                nc.vector.tensor_tensor(prod[:], kview, qb, op=OP.mult)
                sco = cp.tile([P, KZ, AHZ], F32, tag="sco")
                nc.vector.tensor_reduce(sco[:], prod[:],
                                        axis=mybir.AxisListType.X, op=OP.add)

                # ---- bias2 terms
                mu = cp.tile([P, KZ], F32, tag="mu")
                nc.vector.tensor_scalar_mul(mu[:], coll[:, :, 8], 1.0)
                msq = cp.tile([P, KZ], F32, tag="msq")
                nc.vector.tensor_tensor(msq[:], mu[:], mu[:], op=OP.mult)
                var = cp.tile([P, KZ], F32, tag="var")
                nc.vector.scalar_tensor_tensor(var[:], ssq[:], 1.0 / IFZ,
                                               msq[:], op0=OP.mult,
                                               op1=OP.subtract)
                nc.vector.tensor_scalar(var[:], var[:], EPS, None, op0=OP.add)
                rstd = _newton_rsqrt(nc, cp, var[:], KZ, "rsb")
                t1b = cp.tile([P, KZ, AHZ], F32, tag="t1b")
                nc.vector.tensor_tensor(
                    t1b[:], coll[:, :, 8:9].to_broadcast([P, KZ, AHZ]),
                    sg_r[:, None, :].to_broadcast([P, KZ, AHZ]), op=OP.mult)
                t2b = cp.tile([P, KZ, AHZ], F32, tag="t2b")
                nc.vector.tensor_tensor(t2b[:], coll[:, :, 0:AHZ], t1b[:],
                                        op=OP.subtract)
                nc.vector.tensor_tensor(
                    t2b[:], t2b[:], rstd[:, :, None].to_broadcast([P, KZ, AHZ]),
                    op=OP.mult)
                nc.vector.tensor_tensor(sco[:], sco[:], t2b[:], op=OP.add)
                nc.vector.tensor_tensor(
                    sco[:], sco[:], tb_r[:, None, :].to_broadcast([P, KZ, AHZ]),
                    op=OP.add)

                # ---- softmax over k (no max subtraction; scores are small)
                ee = cp.tile([P, KZ, AHZ], BF16, tag="ee")
                nc.scalar.activation(ee[:], sco[:], AF.Exp)
                rsum = cp.tile([P, AHZ], F32, tag="rsum")
                nc.vector.tensor_reduce(rsum[:],
                                        ee[:].rearrange("p k h -> p h k"),
                                        axis=mybir.AxisListType.X, op=OP.add)
                rinv = cp.tile([P, AHZ], F32, tag="rinv")
                nc.vector.reciprocal(rinv[:], rsum[:])

                # ---- weighted V
                vview = kvg[:, :, HF:2 * HF].rearrange("p k (h f) -> p k h f",
                                                       h=AHZ)
                wvt = cp.tile([P, KZ, AHZ, AFZ], BF16, tag="wvt")
                nc.vector.tensor_tensor(
                    wvt[:], vview,
                    ee[:, :, :, None].to_broadcast([P, KZ, AHZ, AFZ]),
                    op=OP.mult)
                au = cp.tile([P, AHZ, AFZ], F32, tag="au")
                nc.vector.tensor_reduce(au[:],
                                        wvt[:].rearrange("p k h f -> p h f k"),
                                        axis=mybir.AxisListType.X, op=OP.add)

                # ---- att = au * gate * rinv
                gsc = cp.tile([P, AHZ, AFZ], BF16, tag="gsc")
                nc.vector.tensor_tensor(
                    gsc[:], gate[:].rearrange("p (h f) -> p h f", h=AHZ),
                    rinv[:, :, None].to_broadcast([P, AHZ, AFZ]), op=OP.mult)
                att = cp.tile([P, HF], BF16, tag="att")
                nc.vector.tensor_tensor(
                    att[:].rearrange("p (h f) -> p h f", h=AHZ), au[:], gsc[:],
                    op=OP.mult)

                # ---- back matmul
                attT = cp.tile([P, 2, P], BF16, tag="attT")
                nc.sync.dma_start_transpose(attT[:], att[:])
                bout = cps.tile([P, IFZ], F32, tag="bout")
                for c in range(2):
                    nc.tensor.matmul(bout[:], attT[:, c, :], wbackb[:, c, :],
                                     start=(c == 0), stop=(c == 1))

                # ---- residual + final layernorm
                res = cp.tile([P, IFZ], F32, tag="res")
                nc.vector.scalar_tensor_tensor(res[:], x1rc[:, t4, :],
                                               math.sqrt(2.0), bout[:],
                                               op0=OP.mult, op1=OP.add)
                nc.vector.tensor_tensor(res[:], res[:], bback_r, op=OP.add)
                smean = cp.tile([P, 1], F32, tag="smean")
                nc.vector.tensor_reduce(smean[:], res[:],
                                        axis=mybir.AxisListType.X, op=OP.add)
                scr = cp.tile([P, IFZ], BF16, tag="scr")
                ssf = cp.tile([P, 1], F32, tag="ssf")
                nc.scalar.activation(scr[:], res[:], AF.Square,
                                     accum_out=ssf[:])
                meanf = cp.tile([P, 1], F32, tag="meanf")
                nc.vector.tensor_scalar_mul(meanf[:], smean[:], 1.0 / IFZ)
                msqf = cp.tile([P, 1], F32, tag="msqf")
                nc.vector.tensor_tensor(msqf[:], meanf[:], meanf[:],
                                        op=OP.mult)
                varf = cp.tile([P, 1], F32, tag="varf")
                nc.vector.scalar_tensor_tensor(varf[:], ssf[:], 1.0 / IFZ,
                                               msqf[:], op0=OP.mult,
                                               op1=OP.subtract)
                nc.vector.tensor_scalar(varf[:], varf[:], EPS, None,
                                        op0=OP.add)
                rstdf = _newton_rsqrt(nc, cp, varf[:], 1, "rsf")
                nbias = cp.tile([P, 1], F32, tag="nbias")
                nc.vector.scalar_tensor_tensor(nbias[:], meanf[:], -1.0,
                                               rstdf[:], op0=OP.mult,
                                               op1=OP.mult)
                xn = cp.tile([P, IFZ], F32, tag="xn")
                nc.scalar.activation(xn[:], res[:], AF.Identity,
                                     scale=rstdf[:], bias=nbias[:])
                nc.vector.tensor_tensor(outw[:, t4, :], xn[:], lng_r,
                                        op=OP.mult)
                nc.vector.tensor_tensor(outw[:, t4, :], outw[:, t4, :], lnb_r,
                                        op=OP.add)
                if t4 == CH - 1:
                    nc.sync.dma_start(
                        out[(t - t4) * P:(t + 1) * P, :]
                        .rearrange("(j p) f -> p j f", p=P), outw[:])

    nc.compile()
    return nc


_NC_CACHE = {}


def _get_nc(n_pad=NP, n_shard=NSH, n_cores=N_CORES):
    key = (n_pad, n_shard, n_cores)
    if key not in _NC_CACHE:
        _NC_CACHE[key] = build_nc(n_cores)
    return _NC_CACHE[key]


def _red(x):
    return (x - 2 * math.pi * np.round(x / (2 * math.pi))).astype(np.float32)


def _pos64(pos):
    """[cos-args | signed-sin-args]: Sin of this gives [cos | ssin] where
    ssin[f<16] = -sin, ssin[f>=16] = +sin."""
    n = pos.shape[0]
    o = np.zeros((n, 2 * AFZ), np.float32)
    o[:, 0:AFZ] = _red(pos + math.pi / 2)
    o[:, AFZ:AFZ + 16] = _red(pos[:, 0:16] + math.pi)
    o[:, AFZ + 16:2 * AFZ] = _red(pos[:, 16:32])
    return o


def make_in_maps(x_1, x_2, pos_emb, edge_index, Wq, Wk, Wv, Wb, bln_g, bln_b,
                 Wg, bg, Wback, bback, ln1_g, ln1_b, n_cores=N_CORES):
    x_1 = np.asarray(x_1, np.float32)
    x_2 = np.asarray(x_2, np.float32)
    pos_emb = np.asarray(pos_emb, np.float32)
    edge_index = np.asarray(edge_index).astype(np.int32)
    n = x_1.shape[0]
    n_shard = n // n_cores     # 2500

    # global tensors (shared by all cores)
    x1p = np.zeros((NP, IFZ), np.float32)
    x1p[:n] = x_1
    x1t = np.ascontiguousarray(
        x1p.T.reshape(2, P, NP), dtype=BF)           # x1t[c, f, n]
    p64 = np.zeros((NP, 2 * AFZ), np.float32)
    p64[:n] = _pos64(pos_emb)
    posf = np.ascontiguousarray(
        p64.reshape(NT1, P, 2 * AFZ).transpose(1, 0, 2))  # [p, t, 64]

    s = 1.0 / math.sqrt(AFZ)
    wkv = np.concatenate([np.asarray(Wk), np.asarray(Wv)], axis=1) \
        .reshape(2, P, 2 * HF).astype(BF)
    wkv = np.ascontiguousarray(wkv.transpose(1, 0, 2))   # [f, c, 512]
    wqg = np.concatenate([np.asarray(Wq) * s, np.asarray(Wg)], axis=1) \
        .reshape(2, P, 2 * HF).astype(BF)
    wqg = np.ascontiguousarray(wqg.transpose(1, 0, 2))
    wb = np.zeros((IFZ, 16), np.float32)
    wb[:, 0:AHZ] = np.asarray(bln_g)[:, None] * np.asarray(Wb)
    wb[:, AHZ] = 1.0 / IFZ
    wb16 = np.ascontiguousarray(
        wb.reshape(2, P, 16).astype(BF).transpose(1, 0, 2))
    wbk = np.asarray(Wback).reshape(2, P, IFZ).astype(BF)
    wback = np.ascontiguousarray(wbk.transpose(1, 0, 2))
    vecs = np.zeros((1, 5 * IFZ + 16), np.float32)
    vecs[0, 0:IFZ] = np.asarray(bg)
    vecs[0, IFZ:2 * IFZ] = np.asarray(ln1_g)
    vecs[0, 2 * IFZ:3 * IFZ] = np.asarray(ln1_b)
    vecs[0, 3 * IFZ:4 * IFZ] = np.asarray(bback)
    vecs[0, 5 * IFZ:5 * IFZ + 8] = np.asarray(bln_g) @ np.asarray(Wb)
    vecs[0, 5 * IFZ + 8:5 * IFZ + 16] = np.asarray(bln_b) @ np.asarray(Wb)

    common = dict(x1t=x1t, posf=posf, wkv=wkv, wqg=wqg, wb16=wb16,
                  wback=wback, vecs=vecs)

    in_maps = []
    for c in range(n_cores):
        lo = c * n_shard
        # poso [p, t, 64]
        po = np.zeros((NSH, 2 * AFZ), np.float32)
        po[:n_shard] = _pos64(pos_emb[lo:lo + n_shard])
        poso = np.ascontiguousarray(
            po.reshape(NT2, P, 2 * AFZ).transpose(1, 0, 2))
        # x2t [f, c2, t, k, m]
        x2s = np.zeros((NSH, KZ, IFZ), np.float32)
        x2s[:n_shard] = x_2[lo:lo + n_shard]
        # [t, m, k, c2, f] -> [f, c2, t, k, m]
        x2r = x2s.reshape(NT2, P, KZ, 2, P).astype(BF)
        x2tc = np.ascontiguousarray(x2r.transpose(4, 3, 0, 2, 1))
        # eit [p, t, k]
        ei = np.zeros((NSH, KZ), np.int32)
        ei[:n_shard] = edge_index[lo:lo + n_shard]
        eit = np.ascontiguousarray(
            ei.reshape(NT2, P, KZ).transpose(1, 0, 2))
        # x1rt [p, t, 256]
        x1r = np.zeros((NSH, IFZ), np.float32)
        x1r[:n_shard] = x_1[lo:lo + n_shard]
        x1rt = np.ascontiguousarray(
            x1r.reshape(NT2, P, IFZ).transpose(1, 0, 2))
        # x1qo [f, c2, m] (own shard, feature-major)
        x1qo = np.ascontiguousarray(
            x1r.T.reshape(2, P, NSH).transpose(1, 0, 2).astype(BF))
        m = dict(common)
        m.update(poso=poso, x2t=x2tc, eit=eit, x1rt=x1rt, x1qo=x1qo)
        in_maps.append(m)
    return in_maps, NP, n_shard


def kernel(**inputs):
    x_1 = np.asarray(inputs["x_1"], np.float32)
    n = x_1.shape[0]
    n_cores = N_CORES
    n_shard = n // n_cores
    in_maps, _, _ = make_in_maps(**inputs)
    nc = _get_nc(NP, NSH, n_cores)
    res = run_bass_kernel_spmd(nc, in_maps, core_ids=list(range(n_cores)),
                               trace=False)
    out = np.concatenate(
        [res.results[c]["out"][:n_shard] for c in range(n_cores)], axis=0)
    return out[:n].astype(np.float32)
